# revision 5
# baseline (speedup 1.0000x reference)
"""GCN layer (gather + segment-sum + linear) on 8 Trainium2 NeuronCores.

Reference computation:
    agg = segment_sum(x[src], dst, num_segments=N)   # (N, 128)
    out = agg @ W + b                                # (N, 128)

Strategy
--------
dst nodes are partitioned across the 8 cores (graph parallel). On the host,
each dst is assigned to a (core, slot) via a degree-sorted snake deal so
that "windows" of 64 slots receive a near-equal number of incoming edges.
Each core processes its 12.5K slots in blocks of 512 (one fp32 PSUM bank).

x is shipped as fp16; src ids are split into 4 equal chunks so the fast
`dma_gather` Q7 ucode (int16 indices) can address them. Per block the core:

  1. runs 4 dma_gather calls (one per src chunk) fetching the fp16 rows
     x[src] of all the block's edges into SBUF, 128 edges per "tile"
     ([128 partitions x 128 feats]); tiles are (window, chunk)-pure,
  2. for each tile runs one TensorE matmul
         psum[:, 64w : 64w+64] += Xg_tile.T @ S_tile
     where S_tile is a host-built one-hot [128 x 64] fp16 matrix
     (S[e, j] = 1 iff edge e's dst sits at column j of window w).
     This is the whole segment-sum at ~1 PE cycle per edge. Edges
     overflowing their window's tiles go to per-(block, chunk)
     "straggler" tiles with a full-width [128 x 512] S,
  3. applies the linear layer in fp32: out[slot,:] = agg[:,slot].T @ W + b,
  4. DMAs the 512 fp32 output rows to HBM.

The host un-permutes the per-core slot outputs back to node order.
Everything is data-parallel; no collectives. The only fp16 rounding is on
x itself (~5e-4 relative error); all accumulation is fp32.
"""

import math

import numpy as np

import concourse.bacc as bacc
import concourse.bass as bass
import concourse.mybir as mybir
from concourse import bass_utils
from concourse.tile import TileContext

NCORES = 8
PART = 128
BLK = 512           # dst slots per block (one PSUM bank of fp32)
WIN = 64            # dst slots per window (normal S tile width)
NCH = 4             # src chunks (int16 index reach)
D = 128             # feature dim (both in and out)

F16 = mybir.dt.float16
F32 = mybir.dt.float32
I16 = mybir.dt.int16

# ---------------------------------------------------------------------------
# Post-compile fixup: this walrus build rejects >1 sync-wait per instruction.
# Hoist extras onto single-wait NOPs inserted just before the offender on the
# same engine queue.
# ---------------------------------------------------------------------------


def _sanitize_waits(nc):
    for bb in nc.m.functions[0].blocks:
        insts = list(bb.instructions)
        out = []
        changed = False
        for inst in insts:
            si = inst.sync_info
            waits = list(si.on_wait) if si is not None and si.on_wait else []
            if len(waits) > 1:
                changed = True
                eng = inst.engine
                for w in waits[:-1]:
                    nop = nc.engines[eng].nop(nofuse=True, hint="wsplit")
                    cb = nc.cur_bb.bb
                    cl = list(cb.instructions)
                    assert cl and cl[-1].name == nop.ins.name
                    cb.instructions = cl[:-1]
                    nop.ins.sync_info = mybir.SyncInfo(
                        on_wait=[w], on_update=[]
                    )
                    out.append(nop.ins)
                inst.sync_info = mybir.SyncInfo(
                    on_wait=[waits[-1]], on_update=list(si.on_update or [])
                )
            out.append(inst)
        if changed:
            bb.instructions = out


# ---------------------------------------------------------------------------
# Static per-core schedule (identical for all cores; SPMD requirement)
# ---------------------------------------------------------------------------


def plan_schedule(npc):
    """For each block: (nslots, [(win_cap_slots, win_ntiles_per_chunk)...])."""
    blocks = []
    for b0 in range(0, npc, BLK):
        ns = min(BLK, npc - b0)
        wins = []
        for w0 in range(0, ns, WIN):
            cap = min(WIN, ns - w0)
            # per-chunk tiles so that capacity ~= expected load (cap*16/NCH)
            nt = max(1, int(round(cap * 16.0 / NCH / 128.0)))
            wins.append((cap, nt))
        blocks.append((ns, wins))
    return blocks


# ---------------------------------------------------------------------------
# Host-side packing
# ---------------------------------------------------------------------------


def pack_inputs(x, edge_index, W, b):
    x = np.asarray(x)
    W = np.asarray(W, dtype=np.float32)
    b = np.asarray(b, dtype=np.float32)
    N, Din = x.shape
    assert Din == D
    E = edge_index.shape[1]
    assert N % NCORES == 0
    npc = N // NCORES
    blocks = plan_schedule(npc)
    nblk = len(blocks)
    CH = int(math.ceil(N / NCH))
    assert CH <= 32768, "src chunk exceeds int16 reach"

    src = np.asarray(edge_index[0], dtype=np.int64)
    dst = np.asarray(edge_index[1], dtype=np.int64)

    # ---- balanced dst -> (core, slot) assignment (snake deal by degree) ----
    deg = np.bincount(dst, minlength=N)
    order = np.argsort(-deg, kind="stable")

    win_caps, win_core, win_slotbase = [], [], []
    for c in range(NCORES):
        for bi, (ns, wins) in enumerate(blocks):
            for w, (cap, _nt) in enumerate(wins):
                win_caps.append(cap)
                win_core.append(c)
                win_slotbase.append(bi * BLK + w * WIN)
    win_caps = np.asarray(win_caps)
    win_core = np.asarray(win_core)
    win_slotbase = np.asarray(win_slotbase)

    core_of_dst = np.empty(N, np.int64)
    slot_of_dst = np.empty(N, np.int64)
    ptr = 0
    for r in range(WIN):
        act = np.flatnonzero(win_caps > r)
        if r % 2 == 1:
            act = act[::-1]
        take = order[ptr:ptr + act.size]
        ptr += act.size
        core_of_dst[take] = win_core[act]
        slot_of_dst[take] = win_slotbase[act] + r
    assert ptr == N

    dst_of_slot = np.empty((NCORES, npc), np.int64)
    dst_of_slot[core_of_dst, slot_of_dst] = np.arange(N)

    # ---- per-edge placement ----
    e_core = core_of_dst[dst]
    e_slot = slot_of_dst[dst]
    e_block = e_slot // BLK
    e_brel = e_slot % BLK
    e_win = e_brel // WIN
    e_col = e_brel % WIN
    e_ch = src // CH

    maxw = max(len(wins) for _, wins in blocks)
    nt_tab = np.zeros((nblk, maxw), np.int64)      # tiles per (win, chunk)
    tbase_tab = np.zeros((nblk, maxw), np.int64)   # tile base within chunk
    NT1 = np.zeros(nblk, np.int64)                 # normal tiles per chunk
    for bi, (ns, wins) in enumerate(blocks):
        t = 0
        for w, (cap, nt) in enumerate(wins):
            nt_tab[bi, w] = nt
            tbase_tab[bi, w] = t
            t += nt
        NT1[bi] = t

    # rank edges within (core, block, window, chunk)
    gid = (((e_core * nblk + e_block) * maxw + e_win) * NCH) + e_ch
    order_e = np.argsort(gid, kind="stable")
    gs = gid[order_e]
    _, start_idx, counts = np.unique(gs, return_index=True, return_counts=True)
    rank = np.arange(E) - np.repeat(start_idx, counts)

    sc = e_core[order_e]
    sb = e_block[order_e]
    sw = e_win[order_e]
    sq = e_ch[order_e]
    scol = e_col[order_e]
    sbrel = e_brel[order_e]
    ssrc = (src - e_ch * CH)[order_e]   # chunk-relative src, < 32768

    cap_e = 128 * nt_tab[sb, sw]
    norm = rank < cap_e
    ov = ~norm

    # straggler ranking within (core, block, chunk)
    gid2 = (sc[ov] * nblk + sb[ov]) * NCH + sq[ov]
    if gid2.size:
        o2 = np.argsort(gid2, kind="stable")
        gid2s = gid2[o2]
        _, start2, counts2 = np.unique(
            gid2s, return_index=True, return_counts=True
        )
        rank2s = np.arange(gid2s.size) - np.repeat(start2, counts2)
        rank2 = np.empty_like(rank2s)
        rank2[o2] = rank2s
        STRAG = int(math.ceil(counts2.max() / 128.0))
    else:
        rank2 = np.zeros(0, np.int64)
        STRAG = 0

    TPC = NT1 + STRAG                  # tiles per chunk-call per block
    TPB = TPC * NCH                    # tiles per block
    tile_off = np.zeros(nblk + 1, np.int64)
    tile_off[1:] = np.cumsum(TPB)
    T_TOT = int(tile_off[-1])
    s64_off = np.zeros(nblk + 1, np.int64)         # normal-tile ordinals
    s64_off[1:] = np.cumsum(NT1 * NCH)
    NT64_TOT = int(s64_off[-1])
    NSTR_TOT = nblk * NCH * STRAG

    idx_all = np.zeros((NCORES, PART, T_TOT), np.int16)
    S64_all = np.zeros((NCORES, PART, NT64_TOT * WIN), np.float16)
    S512_all = (
        np.zeros((NCORES, PART, NSTR_TOT * BLK), np.float16) if STRAG else None
    )

    # normal edges: tile index within block = q*TPC + tbase(w) + rank//128
    tin_n = sq[norm] * TPC[sb[norm]] + tbase_tab[sb[norm], sw[norm]] \
        + rank[norm] // 128
    part_n = rank[norm] % 128
    tile_n = tile_off[sb[norm]] + tin_n
    idx_all[sc[norm], part_n, tile_n] = ssrc[norm].astype(np.int16)
    # S64 ordinal: per-core count of normal tiles before this one
    s64ord = s64_off[sb[norm]] + sq[norm] * NT1[sb[norm]] \
        + tbase_tab[sb[norm], sw[norm]] + rank[norm] // 128
    S64_all[sc[norm], part_n, s64ord * WIN + scol[norm]] = 1.0

    if STRAG:
        ks = rank2 // 128
        part_s = rank2 % 128
        tin_s = sq[ov] * TPC[sb[ov]] + NT1[sb[ov]] + ks
        tile_s = tile_off[sb[ov]] + tin_s
        idx_all[sc[ov], part_s, tile_s] = ssrc[ov].astype(np.int16)
        strord = (sb[ov] * NCH + sq[ov]) * STRAG + ks
        S512_all[sc[ov], part_s, strord * BLK + sbrel[ov]] = 1.0

    # idx16 wrap-replicate layout: for each gather call (block bi, chunk q)
    # covering tiles [t0, t0+TPC): call position i=(t-t0)*128+p lives at
    # int16 column t0*8 + i//16, partition i%16 (replicated to all 8 groups).
    idx16_all = np.zeros((NCORES, PART, T_TOT * 8), np.int16)
    # vectorized: reorder idx_all[c] ([128, T]) into the wrapped layout
    for c in range(NCORES):
        a = idx_all[c]                          # [128, T_TOT]
        # position i within call = (t - t0)*128 + p ; since layout is per
        # tile anyway: for tile t, positions t*128+p map to columns
        # t*8 + (p//16)*... careful: i//16 = (t-t0)*8 + p//16 ; col = t*8+p//16
        # row = p%16. So per tile: [128] -> [8 cols x 16 rows].
        v = a.T.reshape(T_TOT, 8, 16)           # [T, col, row]
        w16 = np.transpose(v, (2, 0, 1)).reshape(16, T_TOT * 8)
        idx16_all[c] = np.tile(w16, (8, 1))

    meta = dict(
        N=N, E=E, npc=npc, nblk=nblk, blocks=blocks, STRAG=STRAG, CH=CH,
        NT1=NT1, TPC=TPC, tile_off=tile_off, s64_off=s64_off,
        T_TOT=T_TOT, NT64_TOT=NT64_TOT, NSTR_TOT=NSTR_TOT,
        dst_of_slot=dst_of_slot,
    )

    x_h = np.ascontiguousarray(x.astype(np.float16))
    brep = np.broadcast_to(b, (PART, D)).copy().astype(np.float32)
    arrs = dict(
        x_h=x_h, W=W.astype(np.float32), brep=brep,
        idx_all=idx_all, idx16_all=idx16_all,
        S64_all=S64_all, S512_all=S512_all,
    )
    return meta, arrs


# ---------------------------------------------------------------------------
# Bass kernel builder (one program shared by all 8 cores)
# ---------------------------------------------------------------------------


def build_nc(meta, repeat=1):
    N = meta["N"]
    npc = meta["npc"]
    nblk = meta["nblk"]
    blocks = meta["blocks"]
    STRAG = meta["STRAG"]
    CH = meta["CH"]
    NT1 = meta["NT1"]
    TPC = meta["TPC"]
    tile_off = meta["tile_off"]
    s64_off = meta["s64_off"]
    T_TOT = meta["T_TOT"]
    NT64_TOT = meta["NT64_TOT"]
    NSTR_TOT = meta["NSTR_TOT"]

    max_tpb = int(max(TPC)) * NCH
    max_nt1 = int(max(NT1))

    nc = bacc.Bacc("TRN2", target_bir_lowering=False, debug=False)
    xh = nc.dram_tensor("xh", [N, D], F16, kind="ExternalInput")
    idx16 = nc.dram_tensor("idx16", [PART, T_TOT * 8], I16, kind="ExternalInput")
    s64 = nc.dram_tensor("s64", [PART, NT64_TOT * WIN], F16, kind="ExternalInput")
    if STRAG:
        s512 = nc.dram_tensor(
            "s512", [PART, NSTR_TOT * BLK], F16, kind="ExternalInput"
        )
    wmat = nc.dram_tensor("wmat", [D, D], F32, kind="ExternalInput")
    brep = nc.dram_tensor("brep", [PART, D], F32, kind="ExternalInput")
    out = nc.dram_tensor("out", [npc, D], F32, kind="ExternalOutput")

    with TileContext(nc) as tc:
        with (
            tc.tile_pool(name="const", bufs=1) as cpool,
            tc.tile_pool(name="io", bufs=3) as iopool,
            tc.tile_pool(name="xgp", bufs=2) as xgp,
            tc.tile_pool(name="outp", bufs=2) as outp,
            tc.tile_pool(name="psum", bufs=2, space="PSUM") as pp,
        ):
            zeros512 = cpool.tile([PART, BLK], F16)
            nc.vector.memset(zeros512[:], 0.0)
            w_sb = cpool.tile([PART, D], F32)
            nc.sync.dma_start(w_sb[:], wmat[:, :])
            brep_sb = cpool.tile([PART, D], F32)
            nc.sync.dma_start(brep_sb[:], brep[:, :])

            for _rep in range(repeat):
                for bi, (ns, wins) in enumerate(blocks):
                    nt1 = int(NT1[bi])
                    tpc = int(TPC[bi])
                    tpb = tpc * NCH
                    toff = int(tile_off[bi])
                    soff = int(s64_off[bi])

                    idx_t = iopool.tile([PART, max_tpb * 8], I16, tag="idx")
                    nc.sync.dma_start(
                        idx_t[:, :tpb * 8],
                        idx16[:, toff * 8:(toff + tpb) * 8],
                    )
                    s64_t = iopool.tile(
                        [PART, max_nt1 * NCH * WIN], F16, tag="s64"
                    )
                    nc.sync.dma_start(
                        s64_t[:, :nt1 * NCH * WIN],
                        s64[:, soff * WIN:(soff + nt1 * NCH) * WIN],
                    )
                    if STRAG:
                        s512_t = iopool.tile(
                            [PART, NCH * STRAG * BLK], F16, tag="s512"
                        )
                        nc.sync.dma_start(
                            s512_t[:],
                            s512[:, bi * NCH * STRAG * BLK:
                                 (bi + 1) * NCH * STRAG * BLK],
                        )

                    xg = xgp.tile([PART, max_tpb * D], F16, tag="xg")
                    for q in range(NCH):
                        lo = q * CH
                        hi = min((q + 1) * CH, N)
                        nc.gpsimd.dma_gather(
                            xg[:, q * tpc * D:(q + 1) * tpc * D].rearrange(
                                "p (t f) -> p t f", t=tpc
                            ),
                            xh[lo:hi, :],
                            idx_t[:, q * tpc * 8:(q + 1) * tpc * 8],
                            tpc * 128,
                            tpc * 128,
                            D,
                            single_packet=False,
                        )

                    agg = pp.tile([PART, BLK], F32, tag="agg")
                    nc.tensor.matmul(
                        agg[:, :], zeros512[:, :D], zeros512[:, :],
                        start=True, stop=False,
                    )
                    n_mm = tpb
                    mm_i = 0
                    for q in range(NCH):
                        ti = q * tpc
                        for w, (cap, ntl) in enumerate(wins):
                            for _k in range(ntl):
                                mm_i += 1
                                nc.tensor.matmul(
                                    agg[:, w * WIN:w * WIN + WIN],
                                    xg[:, ti * D:(ti + 1) * D],
                                    s64_t[:, (q * nt1 + tbase_ord(wins, w)
                                              + _k) * WIN:
                                          (q * nt1 + tbase_ord(wins, w)
                                           + _k + 1) * WIN],
                                    start=False,
                                    stop=(mm_i == n_mm),
                                )
                                ti += 1
                        for k in range(STRAG):
                            mm_i += 1
                            nc.tensor.matmul(
                                agg[:, :],
                                xg[:, ti * D:(ti + 1) * D],
                                s512_t[:, (q * STRAG + k) * BLK:
                                       (q * STRAG + k + 1) * BLK],
                                start=False,
                                stop=(mm_i == n_mm),
                            )
                            ti += 1

                    # linear layer: out[slot, :] = agg[:, slot].T @ W + b
                    agg_sb = outp.tile([PART, BLK], F32, tag="agg_sb")
                    nc.scalar.copy(agg_sb[:, :], agg[:, :])
                    out2 = pp.tile([PART, BLK], F32, tag="out2")
                    out_sb = outp.tile([PART, BLK], F32, tag="out_sb")
                    nout = (ns + 127) // 128
                    for t in range(nout):
                        nc.tensor.matmul(
                            out2[:, t * D:(t + 1) * D],
                            agg_sb[:, t * PART:(t + 1) * PART],
                            w_sb[:, :],
                            start=True, stop=True,
                        )
                        nc.vector.tensor_tensor(
                            out=out_sb[:, t * D:(t + 1) * D],
                            in0=out2[:, t * D:(t + 1) * D],
                            in1=brep_sb[:, :],
                            op=mybir.AluOpType.add,
                        )

                    base = bi * BLK
                    fullt = ns // 128
                    remn = ns % 128
                    if fullt:
                        dview = out[base:base + fullt * 128, :].rearrange(
                            "(t p) f -> p t f", p=PART
                        )
                        sview = out_sb[:, :fullt * D].rearrange(
                            "p (t f) -> p t f", t=fullt
                        )
                        nc.sync.dma_start(dview, sview)
                    if remn:
                        nc.sync.dma_start(
                            out[base + fullt * 128:base + ns, :],
                            out_sb[:remn, fullt * D:(fullt + 1) * D],
                        )
    nc.compile()
    _sanitize_waits(nc)
    return nc


def tbase_ord(wins, w):
    t = 0
    for i in range(w):
        t += wins[i][1]
    return t


# ---------------------------------------------------------------------------
# Entry point
# ---------------------------------------------------------------------------


def make_in_maps(meta, arrs):
    in_maps = []
    for c in range(NCORES):
        m = dict(
            xh=arrs["x_h"],
            idx16=arrs["idx16_all"][c],
            s64=arrs["S64_all"][c],
            wmat=arrs["W"],
            brep=arrs["brep"],
        )
        if meta["STRAG"]:
            m["s512"] = arrs["S512_all"][c]
        in_maps.append(m)
    return in_maps


def assemble_output(meta, results):
    N = meta["N"]
    out_full = np.empty((N, D), np.float32)
    for c in range(NCORES):
        out_full[meta["dst_of_slot"][c]] = results[c]["out"]
    return out_full


def kernel(x, edge_index, W, b):
    meta, arrs = pack_inputs(x, edge_index, W, b)
    nc = build_nc(meta)
    res = bass_utils.run_bass_kernel_spmd(
        nc, make_in_maps(meta, arrs), core_ids=list(range(NCORES))
    )
    return assemble_output(meta, res.results)


# revision 10
# speedup vs baseline: 2137.0668x; 2137.0668x over previous
"""GCN layer (gather + segment-sum + linear) on 8 Trainium2 NeuronCores.

Reference computation:
    agg = segment_sum(x[src], dst, num_segments=N)   # (N, 128)
    out = agg @ W + b                                # (N, 128)

Strategy
--------
dst nodes are partitioned across the 8 cores (graph parallel). On the host,
each dst is assigned to a (core, slot) via a degree-sorted snake deal so
that "windows" of 64 slots receive a near-equal number of incoming edges.
Each core processes its 12.5K slots in blocks of 512 (one fp32 PSUM bank).

x is shipped as fp16; src ids are split into 4 equal chunks so the fast
`dma_gather` Q7 ucode (int16 indices) can address them. Per block the core:

  1. runs 4 dma_gather calls (one per src chunk) fetching the fp16 rows
     x[src] of all the block's edges into SBUF, 128 edges per "tile"
     ([128 partitions x 128 feats]); tiles are (window, chunk)-pure,
  2. for each tile runs one TensorE matmul
         psum[:, 64w : 64w+64] += Xg_tile.T @ S_tile
     where S_tile is a host-built one-hot [128 x 64] fp16 matrix
     (S[e, j] = 1 iff edge e's dst sits at column j of window w).
     This is the whole segment-sum at ~1 PE cycle per edge. Edges
     overflowing their window's tiles go to per-(block, chunk)
     "straggler" tiles with a full-width [128 x 512] S,
  3. applies the linear layer in fp32: out[slot,:] = agg[:,slot].T @ W + b,
  4. DMAs the 512 fp32 output rows to HBM.

The host un-permutes the per-core slot outputs back to node order.
Everything is data-parallel; no collectives. The only fp16 rounding is on
x itself (~5e-4 relative error); all accumulation is fp32.
"""

import math

import numpy as np

import concourse.bacc as bacc
import concourse.bass as bass
import concourse.mybir as mybir
from concourse import bass_utils
from concourse.tile import TileContext

NCORES = 8
PART = 128
BLK = 512           # dst slots per block (one PSUM bank of fp32)
WIN = 64            # dst slots per window (normal S tile width)
NCH = 4             # src chunks (int16 index reach)
D = 128             # feature dim (both in and out)

F16 = mybir.dt.float16
F32 = mybir.dt.float32
F8 = mybir.dt.float8e4
I16 = mybir.dt.int16

# ---------------------------------------------------------------------------
# Post-compile fixup: this walrus build rejects >1 sync-wait per instruction.
# Hoist extras onto single-wait NOPs inserted just before the offender on the
# same engine queue.
# ---------------------------------------------------------------------------


def _sanitize_waits(nc):
    for bb in nc.m.functions[0].blocks:
        insts = list(bb.instructions)
        out = []
        changed = False
        for inst in insts:
            si = inst.sync_info
            waits = list(si.on_wait) if si is not None and si.on_wait else []
            if len(waits) > 1:
                changed = True
                eng = inst.engine
                for w in waits[:-1]:
                    nop = nc.engines[eng].nop(nofuse=True, hint="wsplit")
                    cb = nc.cur_bb.bb
                    cl = list(cb.instructions)
                    assert cl and cl[-1].name == nop.ins.name
                    cb.instructions = cl[:-1]
                    nop.ins.sync_info = mybir.SyncInfo(
                        on_wait=[w], on_update=[]
                    )
                    out.append(nop.ins)
                inst.sync_info = mybir.SyncInfo(
                    on_wait=[waits[-1]], on_update=list(si.on_update or [])
                )
            out.append(inst)
        if changed:
            bb.instructions = out


# ---------------------------------------------------------------------------
# Static per-core schedule (identical for all cores; SPMD requirement)
# ---------------------------------------------------------------------------


def plan_schedule(npc):
    """For each block: (nslots, [(win_cap_slots, win_ntiles_per_chunk)...])."""
    blocks = []
    for b0 in range(0, npc, BLK):
        ns = min(BLK, npc - b0)
        wins = []
        for w0 in range(0, ns, WIN):
            cap = min(WIN, ns - w0)
            # per-chunk tiles so that capacity ~= expected load (cap*16/NCH)
            nt = max(1, int(round(cap * 16.0 / NCH / 128.0)))
            wins.append((cap, nt))
        blocks.append((ns, wins))
    return blocks


# ---------------------------------------------------------------------------
# Host-side packing
# ---------------------------------------------------------------------------


def pack_inputs(x, edge_index, W, b):
    x = np.asarray(x)
    W = np.asarray(W, dtype=np.float32)
    b = np.asarray(b, dtype=np.float32)
    N, Din = x.shape
    assert Din == D
    E = edge_index.shape[1]
    assert N % NCORES == 0
    npc = N // NCORES
    blocks = plan_schedule(npc)
    nblk = len(blocks)
    CH = int(math.ceil(N / NCH))
    assert CH <= 32768, "src chunk exceeds int16 reach"

    src = np.asarray(edge_index[0], dtype=np.int64)
    dst = np.asarray(edge_index[1], dtype=np.int64)

    # ---- balanced dst -> (core, slot) assignment (snake deal by degree) ----
    deg = np.bincount(dst, minlength=N)
    order = np.argsort(-deg, kind="stable")

    win_caps, win_core, win_slotbase = [], [], []
    for c in range(NCORES):
        for bi, (ns, wins) in enumerate(blocks):
            for w, (cap, _nt) in enumerate(wins):
                win_caps.append(cap)
                win_core.append(c)
                win_slotbase.append(bi * BLK + w * WIN)
    win_caps = np.asarray(win_caps)
    win_core = np.asarray(win_core)
    win_slotbase = np.asarray(win_slotbase)

    core_of_dst = np.empty(N, np.int64)
    slot_of_dst = np.empty(N, np.int64)
    ptr = 0
    for r in range(WIN):
        act = np.flatnonzero(win_caps > r)
        if r % 2 == 1:
            act = act[::-1]
        take = order[ptr:ptr + act.size]
        ptr += act.size
        core_of_dst[take] = win_core[act]
        slot_of_dst[take] = win_slotbase[act] + r
    assert ptr == N

    dst_of_slot = np.empty((NCORES, npc), np.int64)
    dst_of_slot[core_of_dst, slot_of_dst] = np.arange(N)

    # ---- per-edge placement ----
    e_core = core_of_dst[dst]
    e_slot = slot_of_dst[dst]
    e_block = e_slot // BLK
    e_brel = e_slot % BLK
    e_win = e_brel // WIN
    e_col = e_brel % WIN
    e_ch = src // CH

    maxw = max(len(wins) for _, wins in blocks)
    nt_tab = np.zeros((nblk, maxw), np.int64)      # tiles per (win, chunk)
    tbase_tab = np.zeros((nblk, maxw), np.int64)   # tile base within chunk
    NT1 = np.zeros(nblk, np.int64)                 # normal tiles per chunk
    for bi, (ns, wins) in enumerate(blocks):
        t = 0
        for w, (cap, nt) in enumerate(wins):
            nt_tab[bi, w] = nt
            tbase_tab[bi, w] = t
            t += nt
        NT1[bi] = t

    # rank edges within (core, block, window, chunk)
    gid = (((e_core * nblk + e_block) * maxw + e_win) * NCH) + e_ch
    order_e = np.argsort(gid, kind="stable")
    gs = gid[order_e]
    _, start_idx, counts = np.unique(gs, return_index=True, return_counts=True)
    rank = np.arange(E) - np.repeat(start_idx, counts)

    sc = e_core[order_e]
    sb = e_block[order_e]
    sw = e_win[order_e]
    sq = e_ch[order_e]
    scol = e_col[order_e]
    sbrel = e_brel[order_e]
    ssrc = (src - e_ch * CH)[order_e]   # chunk-relative src, < 32768

    cap_e = 128 * nt_tab[sb, sw]
    norm = rank < cap_e
    ov = ~norm

    # straggler ranking within (core, block, chunk)
    gid2 = (sc[ov] * nblk + sb[ov]) * NCH + sq[ov]
    if gid2.size:
        o2 = np.argsort(gid2, kind="stable")
        gid2s = gid2[o2]
        _, start2, counts2 = np.unique(
            gid2s, return_index=True, return_counts=True
        )
        rank2s = np.arange(gid2s.size) - np.repeat(start2, counts2)
        rank2 = np.empty_like(rank2s)
        rank2[o2] = rank2s
        STRAG = int(math.ceil(counts2.max() / 128.0))
    else:
        rank2 = np.zeros(0, np.int64)
        STRAG = 0

    TPC = NT1 + STRAG                  # tiles per chunk-call per block
    TPB = TPC * NCH                    # tiles per block
    tile_off = np.zeros(nblk + 1, np.int64)
    tile_off[1:] = np.cumsum(TPB)
    T_TOT = int(tile_off[-1])
    s64_off = np.zeros(nblk + 1, np.int64)         # normal-tile ordinals
    s64_off[1:] = np.cumsum(NT1 * NCH)
    NT64_TOT = int(s64_off[-1])
    NSTR_TOT = nblk * NCH * STRAG

    f8 = mybir.dt.np(F8)
    idx_all = np.zeros((NCORES, PART, T_TOT), np.int16)
    S64_all = np.zeros((NCORES, PART, NT64_TOT * WIN), f8)
    S512_all = (
        np.zeros((NCORES, PART, NSTR_TOT * BLK), f8) if STRAG else None
    )

    # normal edges: tile index within block = q*TPC + tbase(w) + rank//128
    tin_n = sq[norm] * TPC[sb[norm]] + tbase_tab[sb[norm], sw[norm]] \
        + rank[norm] // 128
    part_n = rank[norm] % 128
    tile_n = tile_off[sb[norm]] + tin_n
    idx_all[sc[norm], part_n, tile_n] = ssrc[norm].astype(np.int16)
    # S64 ordinal: per-core count of normal tiles before this one
    s64ord = s64_off[sb[norm]] + sq[norm] * NT1[sb[norm]] \
        + tbase_tab[sb[norm], sw[norm]] + rank[norm] // 128
    S64_all[sc[norm], part_n, s64ord * WIN + scol[norm]] = 1.0

    if STRAG:
        ks = rank2 // 128
        part_s = rank2 % 128
        tin_s = sq[ov] * TPC[sb[ov]] + NT1[sb[ov]] + ks
        tile_s = tile_off[sb[ov]] + tin_s
        idx_all[sc[ov], part_s, tile_s] = ssrc[ov].astype(np.int16)
        strord = (sb[ov] * NCH + sq[ov]) * STRAG + ks
        S512_all[sc[ov], part_s, strord * BLK + sbrel[ov]] = 1.0

    # idx16 wrap-replicate layout: for each gather call (block bi, chunk q)
    # covering tiles [t0, t0+TPC): call position i=(t-t0)*128+p lives at
    # int16 column t0*8 + i//16, partition i%16 (replicated to all 8 groups).
    idx16_all = np.zeros((NCORES, PART, T_TOT * 8), np.int16)
    # vectorized: reorder idx_all[c] ([128, T]) into the wrapped layout
    for c in range(NCORES):
        a = idx_all[c]                          # [128, T_TOT]
        # position i within call = (t - t0)*128 + p ; since layout is per
        # tile anyway: for tile t, positions t*128+p map to columns
        # t*8 + (p//16)*... careful: i//16 = (t-t0)*8 + p//16 ; col = t*8+p//16
        # row = p%16. So per tile: [128] -> [8 cols x 16 rows].
        v = a.T.reshape(T_TOT, 8, 16)           # [T, col, row]
        w16 = np.transpose(v, (2, 0, 1)).reshape(16, T_TOT * 8)
        idx16_all[c] = np.tile(w16, (8, 1))

    meta = dict(
        N=N, E=E, npc=npc, nblk=nblk, blocks=blocks, STRAG=STRAG, CH=CH,
        NT1=NT1, TPC=TPC, tile_off=tile_off, s64_off=s64_off,
        T_TOT=T_TOT, NT64_TOT=NT64_TOT, NSTR_TOT=NSTR_TOT,
        dst_of_slot=dst_of_slot,
    )

    x_h = np.ascontiguousarray(x.astype(np.float16))
    brep = np.broadcast_to(b, (PART, D)).copy().astype(np.float32)
    arrs = dict(
        x_h=x_h, W=W.astype(np.float32), brep=brep,
        idx_all=idx_all, idx16_all=idx16_all,
        S64_all=S64_all, S512_all=S512_all,
    )
    return meta, arrs


# ---------------------------------------------------------------------------
# Bass kernel builder (one program shared by all 8 cores)
# ---------------------------------------------------------------------------


def build_nc(meta, repeat=1, phases=("gather", "mm", "final", "out", "mq")):
    N = meta["N"]
    npc = meta["npc"]
    nblk = meta["nblk"]
    blocks = meta["blocks"]
    STRAG = meta["STRAG"]
    CH = meta["CH"]
    NT1 = meta["NT1"]
    TPC = meta["TPC"]
    tile_off = meta["tile_off"]
    s64_off = meta["s64_off"]
    T_TOT = meta["T_TOT"]
    NT64_TOT = meta["NT64_TOT"]
    NSTR_TOT = meta["NSTR_TOT"]

    max_tpb = int(max(TPC)) * NCH
    max_nt1 = int(max(NT1))

    nc = bacc.Bacc(
        "TRN2", target_bir_lowering=False, debug=False,
        num_swdge_queues=4 if "mq" in phases else 1,
    )
    xh = nc.dram_tensor("xh", [N, D], F16, kind="ExternalInput")
    idx16 = nc.dram_tensor("idx16", [PART, T_TOT * 8], I16, kind="ExternalInput")
    s64 = nc.dram_tensor("s64", [PART, NT64_TOT * WIN], F8, kind="ExternalInput")
    if STRAG:
        s512 = nc.dram_tensor(
            "s512", [PART, NSTR_TOT * BLK], F8, kind="ExternalInput"
        )
    wmat = nc.dram_tensor("wmat", [D, D], F32, kind="ExternalInput")
    brep = nc.dram_tensor("brep", [PART, D], F32, kind="ExternalInput")
    out = nc.dram_tensor("out", [npc, D], F32, kind="ExternalOutput")

    with TileContext(nc) as tc:
        with (
            tc.tile_pool(name="const", bufs=1) as cpool,
            tc.tile_pool(name="io", bufs=3) as iopool,
            tc.tile_pool(name="xgp", bufs=3) as xgp,
            tc.tile_pool(name="outp", bufs=2) as outp,
            tc.tile_pool(name="psum", bufs=2, space="PSUM") as pp,
        ):
            zeros512 = cpool.tile([PART, BLK], F16)
            nc.vector.memset(zeros512[:], 0.0)
            w_sb = cpool.tile([PART, D], F32)
            nc.sync.dma_start(w_sb[:], wmat[:, :])
            brep_sb = cpool.tile([PART, D], F32)
            nc.sync.dma_start(brep_sb[:], brep[:, :])

            for _rep in range(repeat):
                for bi, (ns, wins) in enumerate(blocks):
                    nt1 = int(NT1[bi])
                    tpc = int(TPC[bi])
                    tpb = tpc * NCH
                    toff = int(tile_off[bi])
                    soff = int(s64_off[bi])

                    idx_t = iopool.tile([PART, max_tpb * 8], I16, tag="idx")
                    nc.sync.dma_start(
                        idx_t[:, :tpb * 8],
                        idx16[:, toff * 8:(toff + tpb) * 8],
                    )
                    s64_t = iopool.tile(
                        [PART, max_nt1 * NCH * WIN], F16, tag="s64"
                    )
                    nc.gpsimd.dma_start(
                        s64_t[:, :nt1 * NCH * WIN],
                        s64[:, soff * WIN:(soff + nt1 * NCH) * WIN],
                    )
                    if STRAG:
                        s512_t = iopool.tile(
                            [PART, NCH * STRAG * BLK], F16, tag="s512"
                        )
                        nc.gpsimd.dma_start(
                            s512_t[:],
                            s512[:, bi * NCH * STRAG * BLK:
                                 (bi + 1) * NCH * STRAG * BLK],
                        )

                    xg = xgp.tile([PART, max_tpb * D], F16, tag="xg")
                    if "plain" in phases:
                        nc.sync.dma_start(
                            xg[:, :tpb * D].rearrange("p (t f) -> p t f", t=tpb),
                            xh[:tpb * 128, :].rearrange("(t p) f -> p t f", p=PART),
                        )
                    elif "gather1" in phases:
                        nc.gpsimd.dma_gather(
                            xg[:, :tpb * D].rearrange("p (t f) -> p t f", t=tpb),
                            xh[0:CH, :],
                            idx_t[:, :tpb * 8],
                            tpb * 128,
                            tpb * 128,
                            D,
                            single_packet=False,
                        )
                    else:
                        for q in range(NCH) if "gather" in phases else []:
                            lo = q * CH
                            hi = min((q + 1) * CH, N)
                            nc.gpsimd.dma_gather(
                                xg[:, q * tpc * D:(q + 1) * tpc * D].rearrange(
                                    "p (t f) -> p t f", t=tpc
                                ),
                                xh[lo:hi, :],
                                idx_t[:, q * tpc * 8:(q + 1) * tpc * 8],
                                tpc * 128,
                                tpc * 128,
                                D,
                                single_packet=False,
                                queue_num=q if "mq" in phases else 0,
                            )

                    agg = pp.tile([PART, BLK], F32, tag="agg")
                    if "mm" not in phases:
                        continue
                    nc.tensor.matmul(
                        agg[:, :], zeros512[:, :D], zeros512[:, :],
                        start=True, stop=False,
                    )
                    n_mm = tpb
                    mm_i = 0
                    for q in range(NCH):
                        ti = q * tpc
                        for w, (cap, ntl) in enumerate(wins):
                            for _k in range(ntl):
                                mm_i += 1
                                nc.tensor.matmul(
                                    agg[:, w * WIN:w * WIN + WIN],
                                    xg[:, ti * D:(ti + 1) * D],
                                    s64_t[:, (q * nt1 + tbase_ord(wins, w)
                                              + _k) * WIN:
                                          (q * nt1 + tbase_ord(wins, w)
                                           + _k + 1) * WIN],
                                    start=False,
                                    stop=(mm_i == n_mm),
                                )
                                ti += 1
                        for k in range(STRAG):
                            mm_i += 1
                            nc.tensor.matmul(
                                agg[:, :],
                                xg[:, ti * D:(ti + 1) * D],
                                s512_t[:, (q * STRAG + k) * BLK:
                                       (q * STRAG + k + 1) * BLK],
                                start=False,
                                stop=(mm_i == n_mm),
                            )
                            ti += 1

                    if "final" not in phases:
                        continue
                    # linear layer: out[slot, :] = agg[:, slot].T @ W + b
                    agg_sb = outp.tile([PART, BLK], F32, tag="agg_sb")
                    nc.scalar.copy(agg_sb[:, :], agg[:, :])
                    out2 = pp.tile([PART, BLK], F32, tag="out2")
                    out_sb = outp.tile([PART, BLK], F32, tag="out_sb")
                    nout = (ns + 127) // 128
                    for t in range(nout):
                        nc.tensor.matmul(
                            out2[:, t * D:(t + 1) * D],
                            agg_sb[:, t * PART:(t + 1) * PART],
                            w_sb[:, :],
                            start=True, stop=True,
                        )
                        nc.vector.tensor_tensor(
                            out=out_sb[:, t * D:(t + 1) * D],
                            in0=out2[:, t * D:(t + 1) * D],
                            in1=brep_sb[:, :],
                            op=mybir.AluOpType.add,
                        )

                    if "out" not in phases:
                        continue
                    base = bi * BLK
                    fullt = ns // 128
                    remn = ns % 128
                    if fullt:
                        dview = out[base:base + fullt * 128, :].rearrange(
                            "(t p) f -> p t f", p=PART
                        )
                        sview = out_sb[:, :fullt * D].rearrange(
                            "p (t f) -> p t f", t=fullt
                        )
                        nc.sync.dma_start(dview, sview)
                    if remn:
                        nc.sync.dma_start(
                            out[base + fullt * 128:base + ns, :],
                            out_sb[:remn, fullt * D:(fullt + 1) * D],
                        )
    nc.compile()
    _sanitize_waits(nc)
    return nc


def tbase_ord(wins, w):
    t = 0
    for i in range(w):
        t += wins[i][1]
    return t


# ---------------------------------------------------------------------------
# Entry point
# ---------------------------------------------------------------------------


def make_in_maps(meta, arrs):
    in_maps = []
    for c in range(NCORES):
        m = dict(
            xh=arrs["x_h"],
            idx16=arrs["idx16_all"][c],
            s64=arrs["S64_all"][c],
            wmat=arrs["W"],
            brep=arrs["brep"],
        )
        if meta["STRAG"]:
            m["s512"] = arrs["S512_all"][c]
        in_maps.append(m)
    return in_maps


def assemble_output(meta, results):
    N = meta["N"]
    out_full = np.empty((N, D), np.float32)
    for c in range(NCORES):
        out_full[meta["dst_of_slot"][c]] = results[c]["out"]
    return out_full


def kernel(x, edge_index, W, b):
    meta, arrs = pack_inputs(x, edge_index, W, b)
    nc = build_nc(meta)
    res = bass_utils.run_bass_kernel_spmd(
        nc, make_in_maps(meta, arrs), core_ids=list(range(NCORES))
    )
    return assemble_output(meta, res.results)
